# revision 30
# baseline (speedup 1.0000x reference)
"""AttnCutLoss on 8 Trainium2 NeuronCores (pure data parallel over batch).

loss = -sum_{b,j} log(output[b,j]) * q[b,j] / B,  q = softmax_j(r/tau),
r[b,j] = 2*csum[b,j] / (j+1 + T[b])   (harmonic-mean F1 identity),
csum = cumsum_j(labels), T = total relevant per row.

z = r/tau lies in [0, 1/tau], so softmax needs no max-subtraction:
per row, loss_b = -sum(ln(out)*e^z)/sum(e^z).

Device mapping (per core, 8 tiles of [128 rows x 2048]):
  labels ship as uint8 (lossless), output as float16.  The per-element
  1/(k+T[b]) factor comes from a host-built constant table
  RTAB[T, j] = (2/tau)/(j+1+T) (float16, [2049, 2048]) fetched per tile
  with an indirect row-gather keyed by T.

Measured engine rates (steady state, [128,2048] f16 tiles):
  DVE : scan 4.4us, tensor_tensor 1.22us (2x), STT 2.26us (1x only!)
  ACT : activation pass 2.0us
  Pool: ~0.19 efficiency - only used to drive the indirect gather
so the instruction mix keeps TT (not STT) for the multiplies and the
ip accumulation on the ACT Copy pass (ACT has slack vs DVE).

The kernel is software-pipelined in three phases: the z multiply
trails the scan by 2 tiles (so the gather is never waited on) and the
exp/w/copy chain trails by 3, which keeps the DVE scan stream, the
gpsimd gather stream and the ACT stream all running back-to-back
(the unpipelined version lost ~40us of wall time to cross-engine
stalls).  Tile 7's post-scan stages run as two half-width chains so
the end-of-kernel serial tail is halved; its two accumulator columns
(7, 8) are summed on the host.

Measured dead ends (don't redo these): Pool can't run the scan opcode
(codegen ISA check) and is ~5x slower than DVE even for copies; the
scan runs at 2 cycles/elem regardless of dtype or a stride-0 data1;
indirect-DMA compute_op=mult is rejected by the verifier;
InstTensorTensorReduce crashes the runtime; STT runs at 1x (not the
cost model's 4x); engine clocks vary ~20% run to run.

The Bacc activation-table pass is pinned so Exp and Ln share one table
(natural_log_exp_and_others); the default greedy choice alternates two
tables and pays a 1.3us ACT_TABLE_LOAD per activation.
"""

import numpy as np

import bass_rust as _bass_rust
import concourse.bass as bass
import concourse.tile as tile
from concourse import bacc, mybir
from concourse.bass_utils import run_bass_kernel_spmd
from concourse.hw_specs import get_activation_tables

B, L = 8192, 2048
N_CORES = 8
ROWS_PER_CORE = B // N_CORES          # 1024
P = 128                               # SBUF partitions
TILES_PER_CORE = ROWS_PER_CORE // P   # 8
TAU = 0.95
VTAB = L + 1                          # T can be 0..2048

_CACHE = {}


def _pin_act_tables(nc):
    """Keep Exp/Ln only in the combined table so the table-load pass can't
    alternate between the exp-only and ln-only sets."""

    def patched(self):
        has_activation = any(
            isinstance(i, mybir.InstActivation)
            for b in self.main_func.blocks
            for i in b.instructions
        )
        if not has_activation:
            return
        AF = mybir.ActivationFunctionType
        keep = "natural_log_exp_and_others"
        tables = []
        for name, funcs in get_activation_tables(self.m.arch).items():
            if name != keep:
                funcs = {f for f in funcs if f not in (AF.Exp, AF.Ln)}
            tables.append((name, funcs))
        _bass_rust.insert_act_table_loads(self, tables)

    nc.insert_act_table_loads = patched.__get__(nc)


def _build_nc():
    f16 = mybir.dt.float16
    f32 = mybir.dt.float32
    i32 = mybir.dt.int32
    u8 = mybir.dt.uint8
    AF = mybir.ActivationFunctionType
    OP = mybir.AluOpType

    nc = bacc.Bacc("TRN2", target_bir_lowering=False, debug=False)
    _pin_act_tables(nc)
    labels_d = nc.dram_tensor("labels", [ROWS_PER_CORE, L], u8, kind="ExternalInput")
    outp_d = nc.dram_tensor("outp", [ROWS_PER_CORE, L], f16, kind="ExternalInput")
    rtab_d = nc.dram_tensor("rtab", [VTAB, L], f16, kind="ExternalInput")
    # one accumulator column per full tile 0-5, plus two per half-width
    # drain chain: tiles 6,7 run half-width (6a/6b -> 6/7, 7a/7b -> 8/9),
    # merged on the host
    NACC = 10
    ip_d = nc.dram_tensor("ip_out", [P, NACC], f32, kind="ExternalOutput")
    s_d = nc.dram_tensor("s_out", [P, NACC], f32, kind="ExternalOutput")

    with tile.TileContext(nc) as tc:
        with (
            tc.tile_pool(name="io", bufs=4) as iopool,
            tc.tile_pool(name="front", bufs=TILES_PER_CORE) as fpool,
            tc.tile_pool(name="back", bufs=3) as bpool,
            tc.tile_pool(name="res", bufs=1) as rpool,
        ):
            ip_sb = rpool.tile([P, NACC], f32)
            s_sb = rpool.tile([P, NACC], f32)

            # stage-1 products that must stay live until stage 2 runs
            csum_t = [None] * TILES_PER_CORE
            recip_t = [None] * TILES_PER_CORE
            lo_t = [None] * TILES_PER_CORE
            z_t = {}

            def stage1(t):
                rows = slice(t * P, (t + 1) * P)
                # (issuing tile-0/1 loads from the Activation queue was
                # tried to shrink the ~10.5us head - no effect, reverted)
                Hf2 = L // 2
                lab = iopool.tile([P, L], u8)
                if t == 0:
                    # tile 0 is the pipeline head: split the label DMA and
                    # the scan in halves chained via initial=, so the first
                    # scan starts as soon as the first half lands.
                    nc.sync.dma_start(lab[:, :Hf2], labels_d.ap()[rows, :Hf2])
                    nc.sync.dma_start(lab[:, Hf2:], labels_d.ap()[rows, Hf2:])
                else:
                    nc.sync.dma_start(lab[:], labels_d.ap()[rows, :])
                out = iopool.tile([P, L], f16)
                if t == 0:
                    nc.sync.dma_start(out[:, :Hf2], outp_d.ap()[rows, :Hf2])
                    nc.sync.dma_start(out[:, Hf2:], outp_d.ap()[rows, Hf2:])
                else:
                    nc.sync.dma_start(out[:], outp_d.ap()[rows, :])

                # cumsum along the row (u8 in, f16 out: integers <= 2048,
                # exact).  data1 is ignored (op1=bypass) but the ISA still
                # streams it - feed a stride-0 broadcast so only one real
                # input stream hits SBUF.
                csum = fpool.tile([P, L], f16)
                if t == 0:
                    nc.vector.tensor_tensor_scan(
                        csum[:, :Hf2],
                        lab[:, :Hf2],
                        lab[:, 0:1].broadcast_to([P, Hf2]),
                        0.0,
                        OP.add,
                        OP.bypass,
                    )
                    nc.vector.tensor_tensor_scan(
                        csum[:, Hf2:],
                        lab[:, Hf2:],
                        lab[:, 0:1].broadcast_to([P, Hf2]),
                        csum[:, Hf2 - 1 : Hf2],
                        OP.add,
                        OP.bypass,
                    )
                else:
                    nc.vector.tensor_tensor_scan(
                        csum[:],
                        lab[:],
                        lab[:, 0:1].broadcast_to([P, L]),
                        0.0,
                        OP.add,
                        OP.bypass,
                    )
                # T = csum[:, -1] as int32 row index into the reciprocal
                # table (DVE: the Pool engine takes 1.25us even for [128,1],
                # which sat on the scan->gather critical path)
                offs = bpool.tile([P, 1], i32, tag="offs", bufs=8)
                nc.vector.tensor_copy(offs[:], csum[:, L - 1 : L])
                recip = fpool.tile([P, L], f16)
                nc.gpsimd.indirect_dma_start(
                    out=recip[:],
                    out_offset=None,
                    in_=rtab_d.ap(),
                    in_offset=bass.IndirectOffsetOnAxis(ap=offs[:, :1], axis=0),
                )
                # lo = ln(out)  (independent of the scan/gather chain)
                lo = fpool.tile([P, L], f16)
                if t == 0:
                    nc.scalar.activation(lo[:, :Hf2], out[:, :Hf2], AF.Ln)
                    nc.scalar.activation(lo[:, Hf2:], out[:, Hf2:], AF.Ln)
                else:
                    nc.scalar.activation(lo[:], out[:], AF.Ln)
                csum_t[t], recip_t[t], lo_t[t] = csum, recip, lo

            def stageZ(t, c0=0, c1=L, key=None):
                key = key if key is not None else t
                csum, recip = csum_t[t], recip_t[t]
                # z = (2/tau) * csum / (k + T)  (f16 tensor_tensor, DVE 2x)
                z = bpool.tile([P, c1 - c0], f16, tag="z", bufs=6)
                nc.vector.tensor_tensor(
                    out=z[:], in0=csum[:, c0:c1], in1=recip[:, c0:c1], op=OP.mult
                )
                z_t[key] = z

            e_t = {}

            def stage_e(t, c0=0, c1=L, acc=None):
                # e = exp(z slice), s-accum.  Emitted EARLY on the ACT queue
                # (before the same iteration's Ln) so the w stage never
                # waits on an Ln-delayed exp.
                acc = acc if acc is not None else t
                z = z_t[t]
                e = bpool.tile([P, c1 - c0], f16, tag="e", bufs=4)
                nc.scalar.activation(
                    e[:], z[:, c0:c1], AF.Exp, accum_out=s_sb[:, acc : acc + 1]
                )
                e_t[(t, c0)] = e

            def stage_w_act(t, c0=0, c1=L, acc=None):
                # w = e * lo (DVE TT 2x), ip via ACT Copy accumulator.
                acc = acc if acc is not None else t
                e, lo = e_t[(t, c0)], lo_t[t]
                w = bpool.tile([P, c1 - c0], f16, tag="w", bufs=3)
                nc.vector.tensor_tensor(
                    out=w[:], in0=e[:], in1=lo[:, c0:c1], op=OP.mult
                )
                wc = bpool.tile([P, c1 - c0], f16, tag="wc", bufs=2)
                nc.scalar.activation(
                    wc[:], w[:], AF.Copy, accum_out=ip_sb[:, acc : acc + 1]
                )

            def stage_w_dve(t, c0=0, c1=L, acc=None):
                # w multiply with the ip reduction fused on DVE (STT+accum,
                # 1x) so the ACT Copy backlog doesn't gate the kernel tail.
                acc = acc if acc is not None else t
                e, lo = e_t[(t, c0)], lo_t[t]
                w = bpool.tile([P, c1 - c0], f16, tag="wd", bufs=2)
                nc.vector.scalar_tensor_tensor(
                    out=w[:],
                    in0=e[:],
                    scalar=0.0,
                    in1=lo[:, c0:c1],
                    op0=OP.add,
                    op1=OP.mult,
                    accum_out=ip_sb[:, acc : acc + 1],
                )

            # three-phase software pipeline: the z multiply trails the scan
            # by 2 tiles (the gather has ~2 scan-durations to land) and the
            # exp/w/copy chain trails by 3, so no engine stalls mid-stream.
            # (Tighter offsets 1/2 were tried: no measurable benefit.)
            # During the drain the z->exp->w chains of the last tiles
            # ping-pong between DVE and ACT; tiles 5-7 run as half-width
            # chains (finer granularity) with the ip reduction on DVE so
            # ACT only owes the exps at the end.
            Hf = L // 2
            for t in range(TILES_PER_CORE):
                if t >= 4:
                    stage_e(t - 4)
                stage1(t)
                if t >= 3:
                    stageZ(t - 3)
                if t >= 4:
                    if t - 4 >= 4:
                        stage_w_dve(t - 4)
                    else:
                        stage_w_act(t - 4)
            stage_e(4)
            stageZ(5)
            stage_w_dve(4)
            stage_e(5, acc=5)
            stageZ(6)
            stage_w_dve(5, acc=5)
            stage_e(6, 0, Hf, 6)
            stage_e(6, Hf, L, 7)
            stageZ(7)
            stage_w_dve(6, 0, Hf, 6)
            stage_w_dve(6, Hf, L, 7)
            stage_e(7, 0, Hf, 8)
            stage_e(7, Hf, L, 9)
            stage_w_dve(7, 0, Hf, 8)
            stage_w_dve(7, Hf, L, 9)

            nc.sync.dma_start(ip_d.ap(), ip_sb[:])
            nc.sync.dma_start(s_d.ap(), s_sb[:])
    nc.compile()
    return nc


def _get_nc():
    if "nc" not in _CACHE:
        _CACHE["nc"] = _build_nc()
    return _CACHE["nc"]


def _get_rtab():
    if "rtab" not in _CACHE:
        t = np.arange(VTAB, dtype=np.float64)[:, None]
        k = np.arange(1, L + 1, dtype=np.float64)[None, :]
        _CACHE["rtab"] = ((2.0 / TAU) / (k + t)).astype(np.float16)
    return _CACHE["rtab"]


def _make_in_maps(output, labels):
    outp = np.asarray(output, dtype=np.float32).reshape(B, L).astype(np.float16)
    lab = np.asarray(labels).astype(np.uint8)
    rtab = _get_rtab()
    in_maps = []
    for c in range(N_CORES):
        rows = slice(c * ROWS_PER_CORE, (c + 1) * ROWS_PER_CORE)
        in_maps.append(
            {
                "labels": np.ascontiguousarray(lab[rows]),
                "outp": np.ascontiguousarray(outp[rows]),
                "rtab": rtab,
            }
        )
    return in_maps


def _reduce_results(results):
    total = 0.0
    for r in results:
        ip = r["ip_out"].astype(np.float64)
        s = r["s_out"].astype(np.float64)
        # col pairs (6,7), (8,9) are the half-chains of tiles 6,7
        total += float((ip[:, :6] / s[:, :6]).sum())
        for c in (6, 8):
            total += float(
                ((ip[:, c] + ip[:, c + 1]) / (s[:, c] + s[:, c + 1])).sum()
            )
    return np.float32(-total / B)


def kernel(output, labels):
    nc = _get_nc()
    in_maps = _make_in_maps(output, labels)
    res = run_bass_kernel_spmd(nc, in_maps, list(range(N_CORES)))
    return _reduce_results(res.results)



# revision 32
# speedup vs baseline: 1.0190x; 1.0190x over previous
"""AttnCutLoss on 8 Trainium2 NeuronCores (pure data parallel over batch).

loss = -sum_{b,j} log(output[b,j]) * q[b,j] / B,  q = softmax_j(r/tau),
r[b,j] = 2*csum[b,j] / (j+1 + T[b])   (harmonic-mean F1 identity),
csum = cumsum_j(labels), T = total relevant per row.

z = r/tau lies in [0, 1/tau], so softmax needs no max-subtraction:
per row, loss_b = -sum(ln(out)*e^z)/sum(e^z).

Device mapping (per core, 8 tiles of [128 rows x 2048]):
  labels ship as uint8 (lossless), output as float16.  The per-element
  1/(k+T[b]) factor comes from a host-built constant table
  RTAB[T, j] = (2/tau)/(j+1+T) (float16, [2049, 2048]) fetched per tile
  with an indirect row-gather keyed by T.

Measured engine rates (steady state, [128,2048] f16 tiles):
  DVE : scan 4.4us, tensor_tensor 1.22us (2x), STT 2.26us (1x only!)
  ACT : activation pass 2.0us
  Pool: ~0.19 efficiency - only used to drive the indirect gather
so the instruction mix keeps TT (not STT) for the multiplies and the
ip accumulation on the ACT Copy pass (ACT has slack vs DVE).

The kernel is software-pipelined in three phases: the z multiply
trails the scan by 2 tiles (so the gather is never waited on) and the
exp/w/copy chain trails by 3, which keeps the DVE scan stream, the
gpsimd gather stream and the ACT stream all running back-to-back
(the unpipelined version lost ~40us of wall time to cross-engine
stalls).  Tile 7's post-scan stages run as two half-width chains so
the end-of-kernel serial tail is halved; its two accumulator columns
(7, 8) are summed on the host.

Measured dead ends (don't redo these): Pool can't run the scan opcode
(codegen ISA check) and is ~5x slower than DVE even for copies; the
scan runs at 2 cycles/elem regardless of dtype or a stride-0 data1;
indirect-DMA compute_op=mult is rejected by the verifier;
InstTensorTensorReduce crashes the runtime; STT runs at 1x (not the
cost model's 4x); engine clocks vary ~20% run to run.

The Bacc activation-table pass is pinned so Exp and Ln share one table
(natural_log_exp_and_others); the default greedy choice alternates two
tables and pays a 1.3us ACT_TABLE_LOAD per activation.
"""

import numpy as np

import bass_rust as _bass_rust
import concourse.bass as bass
import concourse.tile as tile
from concourse import bacc, mybir
from concourse.bass_utils import run_bass_kernel_spmd
from concourse.hw_specs import get_activation_tables

B, L = 8192, 2048
N_CORES = 8
ROWS_PER_CORE = B // N_CORES          # 1024
P = 128                               # SBUF partitions
TILES_PER_CORE = ROWS_PER_CORE // P   # 8
TAU = 0.95
VTAB = L + 1                          # T can be 0..2048

_CACHE = {}


def _pin_act_tables(nc):
    """Keep Exp/Ln only in the combined table so the table-load pass can't
    alternate between the exp-only and ln-only sets."""

    def patched(self):
        has_activation = any(
            isinstance(i, mybir.InstActivation)
            for b in self.main_func.blocks
            for i in b.instructions
        )
        if not has_activation:
            return
        AF = mybir.ActivationFunctionType
        keep = "natural_log_exp_and_others"
        tables = []
        for name, funcs in get_activation_tables(self.m.arch).items():
            if name != keep:
                funcs = {f for f in funcs if f not in (AF.Exp, AF.Ln)}
            tables.append((name, funcs))
        _bass_rust.insert_act_table_loads(self, tables)

    nc.insert_act_table_loads = patched.__get__(nc)


def _build_nc():
    f16 = mybir.dt.float16
    f32 = mybir.dt.float32
    u16 = mybir.dt.uint16
    u8 = mybir.dt.uint8
    AF = mybir.ActivationFunctionType
    OP = mybir.AluOpType

    nc = bacc.Bacc("TRN2", target_bir_lowering=False, debug=False)
    _pin_act_tables(nc)
    labels_d = nc.dram_tensor("labels", [ROWS_PER_CORE, L], u8, kind="ExternalInput")
    outp_d = nc.dram_tensor("outp", [ROWS_PER_CORE, L], f16, kind="ExternalInput")
    rtab_d = nc.dram_tensor("rtab", [VTAB, L], f16, kind="ExternalInput")
    # one accumulator column per full tile 0-5, plus two per half-width
    # drain chain: tiles 6,7 run half-width (6a/6b -> 6/7, 7a/7b -> 8/9),
    # merged on the host
    NACC = 10
    ip_d = nc.dram_tensor("ip_out", [P, NACC], f32, kind="ExternalOutput")
    s_d = nc.dram_tensor("s_out", [P, NACC], f32, kind="ExternalOutput")

    with tile.TileContext(nc) as tc:
        with (
            tc.tile_pool(name="io", bufs=4) as iopool,
            tc.tile_pool(name="front", bufs=TILES_PER_CORE) as fpool,
            tc.tile_pool(name="back", bufs=3) as bpool,
            tc.tile_pool(name="res", bufs=1) as rpool,
        ):
            ip_sb = rpool.tile([P, NACC], f32)
            s_sb = rpool.tile([P, NACC], f32)

            # stage-1 products that must stay live until stage 2 runs
            csum_t = [None] * TILES_PER_CORE
            recip_t = [None] * TILES_PER_CORE
            lo_t = [None] * TILES_PER_CORE
            z_t = {}

            def stage1(t):
                rows = slice(t * P, (t + 1) * P)
                # (issuing tile-0/1 loads from the Activation queue was
                # tried to shrink the ~10.5us head - no effect, reverted)
                Hf2 = L // 2
                lab = iopool.tile([P, L], u8)
                if t == 0:
                    # tile 0 is the pipeline head: split the label DMA and
                    # the scan in halves chained via initial=, so the first
                    # scan starts as soon as the first half lands.
                    nc.sync.dma_start(lab[:, :Hf2], labels_d.ap()[rows, :Hf2])
                    nc.sync.dma_start(lab[:, Hf2:], labels_d.ap()[rows, Hf2:])
                else:
                    nc.sync.dma_start(lab[:], labels_d.ap()[rows, :])
                out = iopool.tile([P, L], f16)
                if t == 0:
                    nc.sync.dma_start(out[:, :Hf2], outp_d.ap()[rows, :Hf2])
                    nc.sync.dma_start(out[:, Hf2:], outp_d.ap()[rows, Hf2:])
                else:
                    nc.sync.dma_start(out[:], outp_d.ap()[rows, :])

                # cumsum along the row (u8 in, f16 out: integers <= 2048,
                # exact).  data1 is ignored (op1=bypass) but the ISA still
                # streams it - feed a stride-0 broadcast so only one real
                # input stream hits SBUF.
                csum = fpool.tile([P, L], u16)
                if t == 0:
                    nc.vector.tensor_tensor_scan(
                        csum[:, :Hf2],
                        lab[:, :Hf2],
                        lab[:, 0:1].broadcast_to([P, Hf2]),
                        0.0,
                        OP.add,
                        OP.bypass,
                    )
                    nc.vector.tensor_tensor_scan(
                        csum[:, Hf2:],
                        lab[:, Hf2:],
                        lab[:, 0:1].broadcast_to([P, Hf2]),
                        csum[:, Hf2 - 1 : Hf2],
                        OP.add,
                        OP.bypass,
                    )
                else:
                    nc.vector.tensor_tensor_scan(
                        csum[:],
                        lab[:],
                        lab[:, 0:1].broadcast_to([P, L]),
                        0.0,
                        OP.add,
                        OP.bypass,
                    )
                # the scan emits uint16 (counts <= 2048, exact), so its last
                # column doubles as the gather row index directly - no cast
                # hop on the scan->gather critical path
                recip = fpool.tile([P, L], f16)
                nc.gpsimd.indirect_dma_start(
                    out=recip[:],
                    out_offset=None,
                    in_=rtab_d.ap(),
                    in_offset=bass.IndirectOffsetOnAxis(ap=csum[:, L - 1 : L], axis=0),
                )
                # lo = ln(out)  (independent of the scan/gather chain)
                lo = fpool.tile([P, L], f16)
                if t == 0:
                    nc.scalar.activation(lo[:, :Hf2], out[:, :Hf2], AF.Ln)
                    nc.scalar.activation(lo[:, Hf2:], out[:, Hf2:], AF.Ln)
                else:
                    nc.scalar.activation(lo[:], out[:], AF.Ln)
                csum_t[t], recip_t[t], lo_t[t] = csum, recip, lo

            def stageZ(t, c0=0, c1=L, key=None):
                key = key if key is not None else t
                csum, recip = csum_t[t], recip_t[t]
                # z = (2/tau) * csum / (k + T)  (f16 tensor_tensor, DVE 2x)
                z = bpool.tile([P, c1 - c0], f16, tag="z", bufs=6)
                nc.vector.tensor_tensor(
                    out=z[:], in0=csum[:, c0:c1], in1=recip[:, c0:c1], op=OP.mult
                )
                z_t[key] = z

            e_t = {}

            def stage_e(t, c0=0, c1=L, acc=None):
                # e = exp(z slice), s-accum.  Emitted EARLY on the ACT queue
                # (before the same iteration's Ln) so the w stage never
                # waits on an Ln-delayed exp.
                acc = acc if acc is not None else t
                z = z_t[t]
                e = bpool.tile([P, c1 - c0], f16, tag="e", bufs=4)
                nc.scalar.activation(
                    e[:], z[:, c0:c1], AF.Exp, accum_out=s_sb[:, acc : acc + 1]
                )
                e_t[(t, c0)] = e

            def stage_w_act(t, c0=0, c1=L, acc=None):
                # w = e * lo (DVE TT 2x), ip via ACT Copy accumulator.
                acc = acc if acc is not None else t
                e, lo = e_t[(t, c0)], lo_t[t]
                w = bpool.tile([P, c1 - c0], f16, tag="w", bufs=3)
                nc.vector.tensor_tensor(
                    out=w[:], in0=e[:], in1=lo[:, c0:c1], op=OP.mult
                )
                wc = bpool.tile([P, c1 - c0], f16, tag="wc", bufs=2)
                nc.scalar.activation(
                    wc[:], w[:], AF.Copy, accum_out=ip_sb[:, acc : acc + 1]
                )

            def stage_w_dve(t, c0=0, c1=L, acc=None):
                # w multiply with the ip reduction fused on DVE (STT+accum,
                # 1x) so the ACT Copy backlog doesn't gate the kernel tail.
                acc = acc if acc is not None else t
                e, lo = e_t[(t, c0)], lo_t[t]
                w = bpool.tile([P, c1 - c0], f16, tag="wd", bufs=2)
                nc.vector.scalar_tensor_tensor(
                    out=w[:],
                    in0=e[:],
                    scalar=0.0,
                    in1=lo[:, c0:c1],
                    op0=OP.add,
                    op1=OP.mult,
                    accum_out=ip_sb[:, acc : acc + 1],
                )

            # three-phase software pipeline: the z multiply trails the scan
            # by 2 tiles (the gather has ~2 scan-durations to land) and the
            # exp/w/copy chain trails by 3, so no engine stalls mid-stream.
            # (Tighter offsets 1/2 were tried: no measurable benefit.)
            # During the drain the z->exp->w chains of the last tiles
            # ping-pong between DVE and ACT; tiles 5-7 run as half-width
            # chains (finer granularity) with the ip reduction on DVE so
            # ACT only owes the exps at the end.
            Hf = L // 2
            for t in range(TILES_PER_CORE):
                if t >= 4:
                    stage_e(t - 4)
                stage1(t)
                if t >= 3:
                    stageZ(t - 3)
                if t >= 4:
                    if t - 4 >= 2:
                        stage_w_dve(t - 4)
                    else:
                        stage_w_act(t - 4)
            stage_e(4)
            stageZ(5)
            stage_w_dve(4)
            stage_e(5, acc=5)
            stageZ(6)
            stage_w_dve(5, acc=5)
            stage_e(6, 0, Hf, 6)
            stage_e(6, Hf, L, 7)
            stageZ(7)
            stage_w_dve(6, 0, Hf, 6)
            stage_w_dve(6, Hf, L, 7)
            stage_e(7, 0, Hf, 8)
            stage_e(7, Hf, L, 9)
            stage_w_dve(7, 0, Hf, 8)
            stage_w_dve(7, Hf, L, 9)

            nc.sync.dma_start(ip_d.ap(), ip_sb[:])
            nc.sync.dma_start(s_d.ap(), s_sb[:])
    nc.compile()
    return nc


def _get_nc():
    if "nc" not in _CACHE:
        _CACHE["nc"] = _build_nc()
    return _CACHE["nc"]


def _get_rtab():
    if "rtab" not in _CACHE:
        t = np.arange(VTAB, dtype=np.float64)[:, None]
        k = np.arange(1, L + 1, dtype=np.float64)[None, :]
        _CACHE["rtab"] = ((2.0 / TAU) / (k + t)).astype(np.float16)
    return _CACHE["rtab"]


def _make_in_maps(output, labels):
    outp = np.asarray(output, dtype=np.float32).reshape(B, L).astype(np.float16)
    lab = np.asarray(labels).astype(np.uint8)
    rtab = _get_rtab()
    in_maps = []
    for c in range(N_CORES):
        rows = slice(c * ROWS_PER_CORE, (c + 1) * ROWS_PER_CORE)
        in_maps.append(
            {
                "labels": np.ascontiguousarray(lab[rows]),
                "outp": np.ascontiguousarray(outp[rows]),
                "rtab": rtab,
            }
        )
    return in_maps


def _reduce_results(results):
    total = 0.0
    for r in results:
        ip = r["ip_out"].astype(np.float64)
        s = r["s_out"].astype(np.float64)
        # col pairs (6,7), (8,9) are the half-chains of tiles 6,7
        total += float((ip[:, :6] / s[:, :6]).sum())
        for c in (6, 8):
            total += float(
                ((ip[:, c] + ip[:, c + 1]) / (s[:, c] + s[:, c + 1])).sum()
            )
    return np.float32(-total / B)


def kernel(output, labels):
    nc = _get_nc()
    in_maps = _make_in_maps(output, labels)
    res = run_bass_kernel_spmd(nc, in_maps, list(range(N_CORES)))
    return _reduce_results(res.results)



# revision 33
# speedup vs baseline: 1.0801x; 1.0599x over previous
"""AttnCutLoss on 8 Trainium2 NeuronCores (pure data parallel over batch).

loss = -sum_{b,j} log(output[b,j]) * q[b,j] / B,  q = softmax_j(r/tau),
r[b,j] = 2*csum[b,j] / (j+1 + T[b])   (harmonic-mean F1 identity),
csum = cumsum_j(labels), T = total relevant per row.

z = r/tau lies in [0, 1/tau], so softmax needs no max-subtraction:
per row, loss_b = -sum(ln(out)*e^z)/sum(e^z).

Device mapping (per core, 8 tiles of [128 rows x 2048]):
  labels ship as uint8 (lossless), output as float16.  The per-element
  1/(k+T[b]) factor comes from a host-built constant table
  RTAB[T, j] = (2/tau)/(j+1+T) (float16, [2049, 2048]) fetched per tile
  with an indirect row-gather keyed by T.

Measured engine rates (steady state, [128,2048] f16 tiles):
  DVE : scan 4.4us, tensor_tensor 1.22us (2x), STT 2.26us (1x only!)
  ACT : activation pass 2.0us
  Pool: ~0.19 efficiency - only used to drive the indirect gather
so the instruction mix keeps TT (not STT) for the multiplies and the
ip accumulation on the ACT Copy pass (ACT has slack vs DVE).

The kernel is software-pipelined in three phases: the z multiply
trails the scan by 2 tiles (so the gather is never waited on) and the
exp/w/copy chain trails by 3, which keeps the DVE scan stream, the
gpsimd gather stream and the ACT stream all running back-to-back
(the unpipelined version lost ~40us of wall time to cross-engine
stalls).  Tile 7's post-scan stages run as two half-width chains so
the end-of-kernel serial tail is halved; its two accumulator columns
(7, 8) are summed on the host.

Measured dead ends (don't redo these): Pool can't run the scan opcode
(codegen ISA check) and is ~5x slower than DVE even for copies; the
scan runs at 2 cycles/elem regardless of dtype or a stride-0 data1;
indirect-DMA compute_op=mult is rejected by the verifier;
InstTensorTensorReduce crashes the runtime; STT runs at 1x (not the
cost model's 4x); engine clocks vary ~20% run to run.

The Bacc activation-table pass is pinned so Exp and Ln share one table
(natural_log_exp_and_others); the default greedy choice alternates two
tables and pays a 1.3us ACT_TABLE_LOAD per activation.
"""

import numpy as np

import bass_rust as _bass_rust
import concourse.bass as bass
import concourse.tile as tile
from concourse import bacc, mybir
from concourse.bass_utils import run_bass_kernel_spmd
from concourse.hw_specs import get_activation_tables

B, L = 8192, 2048
N_CORES = 8
ROWS_PER_CORE = B // N_CORES          # 1024
P = 128                               # SBUF partitions
TILES_PER_CORE = ROWS_PER_CORE // P   # 8
TAU = 0.95
VTAB = L + 1                          # T can be 0..2048

_CACHE = {}


def _pin_act_tables(nc):
    """Keep Exp/Ln only in the combined table so the table-load pass can't
    alternate between the exp-only and ln-only sets."""

    def patched(self):
        has_activation = any(
            isinstance(i, mybir.InstActivation)
            for b in self.main_func.blocks
            for i in b.instructions
        )
        if not has_activation:
            return
        AF = mybir.ActivationFunctionType
        keep = "natural_log_exp_and_others"
        tables = []
        for name, funcs in get_activation_tables(self.m.arch).items():
            if name != keep:
                funcs = {f for f in funcs if f not in (AF.Exp, AF.Ln)}
            tables.append((name, funcs))
        _bass_rust.insert_act_table_loads(self, tables)

    nc.insert_act_table_loads = patched.__get__(nc)


def _build_nc():
    f16 = mybir.dt.float16
    f32 = mybir.dt.float32
    u16 = mybir.dt.uint16
    u8 = mybir.dt.uint8
    AF = mybir.ActivationFunctionType
    OP = mybir.AluOpType

    nc = bacc.Bacc("TRN2", target_bir_lowering=False, debug=False)
    _pin_act_tables(nc)
    labels_d = nc.dram_tensor("labels", [ROWS_PER_CORE, L], u8, kind="ExternalInput")
    outp_d = nc.dram_tensor("outp", [ROWS_PER_CORE, L], f16, kind="ExternalInput")
    rtab_d = nc.dram_tensor("rtab", [VTAB, L], f16, kind="ExternalInput")
    # one accumulator column per full tile 0-5, plus two per half-width
    # drain chain: tiles 6,7 run half-width (6a/6b -> 6/7, 7a/7b -> 8/9),
    # merged on the host
    NACC = 10
    ip_d = nc.dram_tensor("ip_out", [P, NACC], f32, kind="ExternalOutput")
    s_d = nc.dram_tensor("s_out", [P, NACC], f32, kind="ExternalOutput")

    with tile.TileContext(nc) as tc:
        with (
            tc.tile_pool(name="io", bufs=4) as iopool,
            tc.tile_pool(name="front", bufs=TILES_PER_CORE) as fpool,
            tc.tile_pool(name="back", bufs=3) as bpool,
            tc.tile_pool(name="res", bufs=1) as rpool,
        ):
            ip_sb = rpool.tile([P, NACC], f32)
            s_sb = rpool.tile([P, NACC], f32)

            # stage-1 products that must stay live until stage 2 runs
            csum_t = [None] * TILES_PER_CORE
            recip_t = [None] * TILES_PER_CORE
            lo_t = [None] * TILES_PER_CORE
            z_t = {}

            def stage1(t):
                rows = slice(t * P, (t + 1) * P)
                # (issuing tile-0/1 loads from the Activation queue was
                # tried to shrink the ~10.5us head - no effect, reverted)
                Hf2 = L // 2
                lab = iopool.tile([P, L], u8)
                if t == 0:
                    # tile 0 is the pipeline head: split the label DMA and
                    # the scan in halves chained via initial=, so the first
                    # scan starts as soon as the first half lands.
                    nc.sync.dma_start(lab[:, :Hf2], labels_d.ap()[rows, :Hf2])
                    nc.sync.dma_start(lab[:, Hf2:], labels_d.ap()[rows, Hf2:])
                else:
                    nc.sync.dma_start(lab[:], labels_d.ap()[rows, :])
                out = iopool.tile([P, L], f16)
                if t == 0:
                    nc.sync.dma_start(out[:, :Hf2], outp_d.ap()[rows, :Hf2])
                    nc.sync.dma_start(out[:, Hf2:], outp_d.ap()[rows, Hf2:])
                else:
                    nc.sync.dma_start(out[:], outp_d.ap()[rows, :])

                # cumsum along the row (u8 in, f16 out: integers <= 2048,
                # exact).  data1 is ignored (op1=bypass) but the ISA still
                # streams it - feed a stride-0 broadcast so only one real
                # input stream hits SBUF.
                csum = fpool.tile([P, L], u16)
                if t == 0:
                    nc.vector.tensor_tensor_scan(
                        csum[:, :Hf2],
                        lab[:, :Hf2],
                        lab[:, 0:1].broadcast_to([P, Hf2]),
                        0.0,
                        OP.add,
                        OP.bypass,
                    )
                    nc.vector.tensor_tensor_scan(
                        csum[:, Hf2:],
                        lab[:, Hf2:],
                        lab[:, 0:1].broadcast_to([P, Hf2]),
                        csum[:, Hf2 - 1 : Hf2],
                        OP.add,
                        OP.bypass,
                    )
                else:
                    nc.vector.tensor_tensor_scan(
                        csum[:],
                        lab[:],
                        lab[:, 0:1].broadcast_to([P, L]),
                        0.0,
                        OP.add,
                        OP.bypass,
                    )
                # the scan emits uint16 (counts <= 2048, exact), so its last
                # column doubles as the gather row index directly - no cast
                # hop on the scan->gather critical path
                recip = fpool.tile([P, L], f16)
                nc.gpsimd.indirect_dma_start(
                    out=recip[:],
                    out_offset=None,
                    in_=rtab_d.ap(),
                    in_offset=bass.IndirectOffsetOnAxis(ap=csum[:, L - 1 : L], axis=0),
                )
                # lo = ln(out)  (independent of the scan/gather chain)
                lo = fpool.tile([P, L], f16)
                if t == 0:
                    nc.scalar.activation(lo[:, :Hf2], out[:, :Hf2], AF.Ln)
                    nc.scalar.activation(lo[:, Hf2:], out[:, Hf2:], AF.Ln)
                else:
                    nc.scalar.activation(lo[:], out[:], AF.Ln)
                csum_t[t], recip_t[t], lo_t[t] = csum, recip, lo

            def stageZ(t, c0=0, c1=L, key=None):
                key = key if key is not None else t
                csum, recip = csum_t[t], recip_t[t]
                # z = (2/tau) * csum / (k + T)  (f16 tensor_tensor, DVE 2x)
                z = bpool.tile([P, c1 - c0], f16, tag="z", bufs=6)
                nc.vector.tensor_tensor(
                    out=z[:], in0=csum[:, c0:c1], in1=recip[:, c0:c1], op=OP.mult
                )
                z_t[key] = z

            e_t = {}

            def stage_e(t, c0=0, c1=L, acc=None):
                # e = exp(z slice), s-accum.  Emitted EARLY on the ACT queue
                # (before the same iteration's Ln) so the w stage never
                # waits on an Ln-delayed exp.
                acc = acc if acc is not None else t
                z = z_t[t]
                e = bpool.tile([P, c1 - c0], f16, tag="e", bufs=4)
                nc.scalar.activation(
                    e[:], z[:, c0:c1], AF.Exp, accum_out=s_sb[:, acc : acc + 1]
                )
                e_t[(t, c0)] = e

            def stage_w_act(t, c0=0, c1=L, acc=None):
                # w = e * lo (DVE TT 2x), ip via ACT Copy accumulator.
                acc = acc if acc is not None else t
                e, lo = e_t[(t, c0)], lo_t[t]
                w = bpool.tile([P, c1 - c0], f16, tag="w", bufs=3)
                nc.vector.tensor_tensor(
                    out=w[:], in0=e[:], in1=lo[:, c0:c1], op=OP.mult
                )
                wc = bpool.tile([P, c1 - c0], f16, tag="wc", bufs=2)
                nc.scalar.activation(
                    wc[:], w[:], AF.Copy, accum_out=ip_sb[:, acc : acc + 1]
                )

            def stage_w_dve(t, c0=0, c1=L, acc=None):
                # w multiply with the ip reduction fused on DVE (STT+accum,
                # 1x) so the ACT Copy backlog doesn't gate the kernel tail.
                acc = acc if acc is not None else t
                e, lo = e_t[(t, c0)], lo_t[t]
                w = bpool.tile([P, c1 - c0], f16, tag="wd", bufs=2)
                nc.vector.scalar_tensor_tensor(
                    out=w[:],
                    in0=e[:],
                    scalar=0.0,
                    in1=lo[:, c0:c1],
                    op0=OP.add,
                    op1=OP.mult,
                    accum_out=ip_sb[:, acc : acc + 1],
                )

            # three-phase software pipeline: the z multiply trails the scan
            # by 2 tiles (the gather has ~2 scan-durations to land) and the
            # exp/w/copy chain trails by 3, so no engine stalls mid-stream.
            # (Tighter offsets 1/2 were tried: no measurable benefit.)
            # During the drain the z->exp->w chains of the last tiles
            # ping-pong between DVE and ACT; tiles 5-7 run as half-width
            # chains (finer granularity) with the ip reduction on DVE so
            # ACT only owes the exps at the end.
            Hf = L // 2
            for t in range(TILES_PER_CORE):
                if t >= 4:
                    stage_e(t - 4)
                stage1(t)
                if t >= 4:
                    if t - 4 >= 2:
                        stage_w_dve(t - 4)
                    else:
                        stage_w_act(t - 4)
                if t >= 3:
                    stageZ(t - 3)
            stage_e(4)
            stageZ(5)
            stage_w_dve(4)
            stage_e(5, acc=5)
            stageZ(6)
            stage_w_dve(5, acc=5)
            stage_e(6, 0, Hf, 6)
            stage_e(6, Hf, L, 7)
            stageZ(7)
            stage_w_dve(6, 0, Hf, 6)
            stage_w_dve(6, Hf, L, 7)
            stage_e(7, 0, Hf, 8)
            stage_e(7, Hf, L, 9)
            stage_w_dve(7, 0, Hf, 8)
            stage_w_dve(7, Hf, L, 9)

            nc.sync.dma_start(ip_d.ap(), ip_sb[:])
            nc.sync.dma_start(s_d.ap(), s_sb[:])
    nc.compile()
    return nc


def _get_nc():
    if "nc" not in _CACHE:
        _CACHE["nc"] = _build_nc()
    return _CACHE["nc"]


def _get_rtab():
    if "rtab" not in _CACHE:
        t = np.arange(VTAB, dtype=np.float64)[:, None]
        k = np.arange(1, L + 1, dtype=np.float64)[None, :]
        _CACHE["rtab"] = ((2.0 / TAU) / (k + t)).astype(np.float16)
    return _CACHE["rtab"]


def _make_in_maps(output, labels):
    outp = np.asarray(output, dtype=np.float32).reshape(B, L).astype(np.float16)
    lab = np.asarray(labels).astype(np.uint8)
    rtab = _get_rtab()
    in_maps = []
    for c in range(N_CORES):
        rows = slice(c * ROWS_PER_CORE, (c + 1) * ROWS_PER_CORE)
        in_maps.append(
            {
                "labels": np.ascontiguousarray(lab[rows]),
                "outp": np.ascontiguousarray(outp[rows]),
                "rtab": rtab,
            }
        )
    return in_maps


def _reduce_results(results):
    total = 0.0
    for r in results:
        ip = r["ip_out"].astype(np.float64)
        s = r["s_out"].astype(np.float64)
        # col pairs (6,7), (8,9) are the half-chains of tiles 6,7
        total += float((ip[:, :6] / s[:, :6]).sum())
        for c in (6, 8):
            total += float(
                ((ip[:, c] + ip[:, c + 1]) / (s[:, c] + s[:, c + 1])).sum()
            )
    return np.float32(-total / B)


def kernel(output, labels):
    nc = _get_nc()
    in_maps = _make_in_maps(output, labels)
    res = run_bass_kernel_spmd(nc, in_maps, list(range(N_CORES)))
    return _reduce_results(res.results)

